# revision 1
# baseline (speedup 1.0000x reference)
"""Causal multi-head self-attention with RoPE on 8 TRN2 NeuronCores.

Sharding: batch(4) x head-group(2) -> 8 cores. Core c handles batch c//2 and
heads [8*(c%2), 8*(c%2)+8). Each core computes its partial output projection
(sum over its 8 heads' contribution); the host adds the two head-group
partials per batch. No device collectives needed.

On-chip layout: sequence lives on the free dimension everywhere.
  - Q^T/K^T [j, s] come straight out of the projection matmuls
    (lhsT = W^T slices, rhs = x^T), RoPE applied with a partition pair-swap
    (stream_shuffle) + precomputed cos/sin tables.
  - scores are computed transposed S^T = K^T.T-contraction -> [k, q] tiles,
    exp on ScalarE (no max subtraction needed: |scores| <= ~15), causal
    masking by adding -1e30 on diagonal tiles before exp.
  - P@V contraction runs over k on partitions; a ones-row appended to V
    makes the softmax denominator fall out of the same matmul (M=65).
  - output projection contracts the 512 head-dims -> partial y^T [1024, s].
"""

import os
import sys
import time

for _p in ("/opt/trn_rl_repo", "/root/.axon_site/_ro/trn_rl_repo"):
    if _p not in sys.path and os.path.isdir(_p):
        sys.path.insert(0, _p)

import numpy as np
import concourse.bass as bass
import concourse.bacc as bacc
import concourse.mybir as mybir
import concourse.tile as tile
from concourse.bass_utils import run_bass_kernel_spmd

F32 = mybir.dt.float32
F32R = mybir.dt.float32r

B, S, D = 4, 2048, 1024
H, DK = 16, 64
HPC = 8            # heads per core
JC = HPC * DK      # 512 head-dims per core
N_CORES = 8
SC = 512           # q-chunk width (moving free dim)
NSC = S // SC      # 4
KT = 128           # k-tile (scores partition dim)
NKT = S // KT      # 16
DT = D // 128      # 8 contraction tiles for projections

# matmul operand dtype: "f32" (exact) or "f32r" (tf32-like, ~4x faster PE)
MM_DTYPE = os.environ.get("KERNEL_MM_DTYPE", "f32")
MERGED_EXP = os.environ.get("KERNEL_MERGED_EXP", "0") == "1"
EBUFS = int(os.environ.get("KV_EBUFS", "5"))
SCBUFS = int(os.environ.get("KV_SCBUFS", "2"))
YBUFS = int(os.environ.get("KV_YBUFS", "2"))
NOMASK = os.environ.get("KV_NOMASK", "0") == "1"
NOROPE = os.environ.get("KV_NOROPE", "0") == "1"
APBUFS = int(os.environ.get("KV_APBUFS", "2"))
XBUFS = int(os.environ.get("KV_XBUFS", "2"))
RBUFS = int(os.environ.get("KV_RBUFS", "1"))
OBUFS = int(os.environ.get("KV_OBUFS", "2"))
WOPRE = os.environ.get("KV_WOPRE", "0") == "1"


_PAIR_SWAP = []
for _i in range(16):
    _PAIR_SWAP += [2 * _i + 1, 2 * _i]


def _emit(nc, tc, mmdt, dram, tag=""):
    """Emit the whole per-core program. `dram` maps name -> DRAM AP."""
    xT = dram["xT"]
    wq, wk, wv, wo = dram["wq"], dram["wk"], dram["wv"], dram["wo"]
    cosE, sinE, maskneg = dram["cosE"], dram["sinE"], dram["maskneg"]
    yT = dram["yT"]

    need_round = mmdt != F32
    EXP = mybir.ActivationFunctionType.Exp

    import contextlib
    with contextlib.ExitStack() as ctx:
        # ---- persistent tiles -------------------------------------------
        per = ctx.enter_context(tc.tile_pool(name=f"per{tag}", bufs=1))
        QT = [per.tile([128, S], mmdt, tag=f"QT{j}{tag}", name=f"QT{j}{tag}") for j in range(4)]
        KTt = [per.tile([128, S], mmdt, tag=f"KT{j}{tag}", name=f"KT{j}{tag}") for j in range(4)]
        vo = [per.tile([128, HPC, 65], mmdt, tag=f"vo{i}{tag}", name=f"vo{i}{tag}") for i in range(NKT)]
        ones_sb = per.tile([128, HPC], F32, tag=f"ones{tag}", name=f"ones{tag}")
        nc.vector.memset(ones_sb, 1.0)
        cos_sb = per.tile([128, S], F32, tag=f"cos{tag}", name=f"cos{tag}")
        sin_sb = per.tile([128, S], F32, tag=f"sin{tag}", name=f"sin{tag}")
        mask_sb = per.tile([128, 128], F32, tag=f"mask{tag}", name=f"mask{tag}")
        wo_pre = None
        if WOPRE:
            wo_pre = per.tile([128, 4, D], mmdt, tag=f"wop{tag}", name=f"wop{tag}")
            wo_r0 = wo.rearrange("(hp p) m -> p hp m", p=128)
            for hp in range(4):
                nc.sync.dma_start(out=wo_pre[:, hp, :], in_=wo_r0[:, hp, :])
        nc.sync.dma_start(out=cos_sb, in_=cosE)
        nc.sync.dma_start(out=sin_sb, in_=sinE)
        nc.sync.dma_start(out=mask_sb, in_=maskneg)

        # ---- phase A: projections ---------------------------------------
        with tc.tile_pool(name=f"pA{tag}", bufs=1) as pa, \
             tc.tile_pool(name=f"pAx{tag}", bufs=1) as pax, \
             tc.tile_pool(name=f"pAt{tag}", bufs=RBUFS) as pat, \
             tc.tile_pool(name=f"pAps{tag}", bufs=APBUFS, space="PSUM") as paps:
            w_sb = {}
            for name, w_ap in (("wq", wq), ("wk", wk), ("wv", wv)):
                wt = pa.tile([128, DT, JC], mmdt, tag=f"{name}{tag}", name=f"{name}{tag}")
                w_r = w_ap.rearrange("(dt p) j -> p dt j", p=128)
                for dt in range(DT):
                    nc.sync.dma_start(out=wt[:, dt, :], in_=w_r[:, dt, :])
                w_sb[name] = wt

            xT_r = xT.rearrange("(dt p) s -> p dt s", p=128)
            for sc in range(NSC):
                ssl = slice(sc * SC, (sc + 1) * SC)
                xc = pax.tile([128, DT, SC], mmdt, tag=f"xc{tag}",
                              name=f"xc{tag}", bufs=XBUFS)
                for dt in range(DT):  # per-dt DMAs spread across queues
                    nc.sync.dma_start(out=xc[:, dt, :],
                                      in_=xT_r[:, dt, ssl])

                # V: out[s, j] tiles, lhsT = x^T [d, s], rhs = Wv^T [d, j]
                for st in range(4):
                    pv = paps.tile([128, JC], F32, tag=f"vps{tag}", name=f"vps{tag}")
                    sl = slice(st * 128, (st + 1) * 128)
                    for dt in range(DT):
                        nc.tensor.matmul(
                            pv, xc[:, dt, sl], w_sb["wv"][:, dt, :],
                            start=(dt == 0), stop=(dt == DT - 1))
                    vt = vo[sc * 4 + st]
                    nc.vector.tensor_copy(
                        vt[:, :, 0:64],
                        pv.rearrange("p (h j) -> p h j", h=HPC))
                    if need_round:
                        # memset on an f32r tile is ISA-invalid; copy from an
                        # f32 ones tile instead (copy rounds to f32r)
                        nc.vector.tensor_copy(
                            vt[:, :, 64:65],
                            ones_sb.rearrange("p (h o) -> p h o", o=1))
                    else:
                        nc.vector.memset(vt[:, :, 64:65], 1.0)

                # Q/K: out = (W row-slice) @ x^T -> [j, s] + RoPE
                for wname, dst in (("wq", QT), ("wk", KTt)):
                    for jt in range(4):
                        ps = paps.tile([128, SC], F32, tag=f"qkps{tag}", name=f"qkps{tag}")
                        jl = slice(jt * 128, (jt + 1) * 128)
                        for dt in range(DT):
                            nc.tensor.matmul(
                                ps, w_sb[wname][:, dt, jl], xc[:, dt, :],
                                start=(dt == 0), stop=(dt == DT - 1))
                        if NOROPE:
                            nc.vector.tensor_copy(dst[jt][:, ssl], ps)
                        else:
                            qs = pat.tile([128, SC], F32, tag=f"ropes{tag}", name=f"ropes{tag}")
                            nc.vector.stream_shuffle(qs, ps, _PAIR_SWAP)
                            qc_t = pat.tile([128, SC], F32, tag=f"ropec{tag}", name=f"ropec{tag}")
                            nc.vector.tensor_mul(qc_t, ps, cos_sb[:, ssl])
                            nc.vector.tensor_mul(qs, qs, sin_sb[:, ssl])
                            nc.vector.tensor_add(dst[jt][:, ssl], qc_t, qs)

        # ---- phase B: attention + output projection ---------------------
        with tc.tile_pool(name=f"pB{tag}", bufs=1) as pb, \
             tc.tile_pool(name=f"pBe{tag}", bufs=EBUFS) as pbe, \
             tc.tile_pool(name=f"pBt{tag}", bufs=2) as pbt, \
             tc.tile_pool(name=f"pBo{tag}", bufs=OBUFS) as pbo, \
             tc.tile_pool(name=f"pBps{tag}", bufs=1, space="PSUM") as pbps, \
             tc.tile_pool(name=f"pBps2{tag}", bufs=2, space="PSUM") as pbps2:
            if WOPRE:
                wo_sb = wo_pre
            else:
                wo_sb = pb.tile([128, 4, D], mmdt, tag=f"wo{tag}", name=f"wo{tag}")
                wo_r = wo.rearrange("(hp p) m -> p hp m", p=128)
                for hp in range(4):
                    nc.sync.dma_start(out=wo_sb[:, hp, :], in_=wo_r[:, hp, :])

            for qc in range(NSC):
                qsl = slice(qc * SC, (qc + 1) * SC)
                oTs = []
                for hp in range(4):
                    pva = pbps.tile([65, SC], F32, tag=f"pva{tag}", name=f"pva{tag}")
                    pvb = pbps.tile([65, SC], F32, tag=f"pvb{tag}", name=f"pvb{tag}")
                    nkt = 4 * qc + 4
                    h0, h1 = 2 * hp, 2 * hp + 1
                    pending = None  # software pipeline: PV lags scores by 1
                    for kt in range(nkt):
                        ksl = slice(kt * KT, (kt + 1) * KT)
                        d = kt - 4 * qc
                        # diagonal tiles: only columns q >= 128*d are causally
                        # valid -- shrink scores/exp/PV to that range; the
                        # boundary 128-wide strip still needs the triangular
                        # mask-add.
                        cs = 128 * d if d > 0 else 0
                        vq = slice(cs, SC)
                        qv = slice(qc * SC + cs, (qc + 1) * SC)
                        if MERGED_EXP:
                            sc2 = pbps2.tile([128, 2, SC], F32, tag=f"sc2{tag}",
                                             name=f"sc2{tag}", bufs=SCBUFS)
                            sca, scb = sc2[:, 0, :], sc2[:, 1, :]
                        else:
                            sca = pbps2.tile([128, SC], F32, tag=f"sca{tag}", name=f"sca{tag}", bufs=SCBUFS)
                            scb = pbps2.tile([128, SC], F32, tag=f"scb{tag}", name=f"scb{tag}", bufs=SCBUFS)
                        nc.tensor.matmul(sca[:, vq], KTt[hp][0:64, ksl],
                                         QT[hp][0:64, qv],
                                         start=True, stop=True,
                                         tile_position=(0, 0))
                        nc.tensor.matmul(scb[:, vq], KTt[hp][64:128, ksl],
                                         QT[hp][64:128, qv],
                                         start=True, stop=True,
                                         tile_position=(64, 0))
                        if d >= 0 and not NOMASK:  # triangular boundary strip
                            bs = slice(cs, cs + 128)
                            nc.vector.tensor_add(sca[:, bs], sca[:, bs],
                                                 mask_sb)
                            nc.vector.tensor_add(scb[:, bs], scb[:, bs],
                                                 mask_sb)
                        if MERGED_EXP:
                            e2 = pbe.tile([128, 2, SC], mmdt, tag=f"e2{tag}",
                                          name=f"e2{tag}")
                            nc.scalar.activation(e2[:, :, vq], sc2[:, :, vq],
                                                 EXP, scale=0.125)
                            ea, eb = e2[:, 0, :], e2[:, 1, :]
                        else:
                            ea = pbe.tile([128, SC], mmdt, tag=f"ea{tag}", name=f"ea{tag}")
                            eb = pbe.tile([128, SC], mmdt, tag=f"eb{tag}", name=f"eb{tag}")
                            nc.scalar.activation(ea[:, vq], sca[:, vq], EXP,
                                                 scale=0.125)
                            nc.scalar.activation(eb[:, vq], scb[:, vq], EXP,
                                                 scale=0.125)
                        if pending is not None:
                            pkt, pea, peb, pvq = pending
                            nc.tensor.matmul(pva[:, pvq], vo[pkt][:, h0, :],
                                             pea[:, pvq],
                                             start=(pkt == 0), stop=False)
                            nc.tensor.matmul(pvb[:, pvq], vo[pkt][:, h1, :],
                                             peb[:, pvq],
                                             start=(pkt == 0), stop=False)
                        pending = (kt, ea, eb, vq)
                    pkt, pea, peb, pvq = pending
                    nc.tensor.matmul(pva[:, pvq], vo[pkt][:, h0, :],
                                     pea[:, pvq],
                                     start=(pkt == 0), stop=True)
                    nc.tensor.matmul(pvb[:, pvq], vo[pkt][:, h1, :],
                                     peb[:, pvq],
                                     start=(pkt == 0), stop=True)
                    # normalize: oT[j, q] = pv[j, q] / denom[q].
                    # All DVE ops must be partition-aligned; the denom row
                    # lives at partition 64, so recip in place (64->64), DMA
                    # the row to partition 0, broadcast to 0:64, multiply at
                    # base 0, and DMA-relocate head B's rows to 64:128.
                    rc = pbt.tile([65, 2, SC], F32, tag=f"rc{tag}", name=f"rc{tag}", bufs=1)
                    nc.vector.reciprocal(rc[64:65, 0, :], pva[64:65, :])
                    nc.vector.reciprocal(rc[64:65, 1, :], pvb[64:65, :])
                    r0 = pbt.tile([1, 2, SC], F32, tag=f"r0{tag}", name=f"r0{tag}", bufs=1)
                    nc.sync.dma_start(out=r0, in_=rc[64:65, :, :])
                    bc = pbt.tile([64, 2, SC], F32, tag=f"bc{tag}", name=f"bc{tag}")
                    nc.gpsimd.partition_broadcast(bc[:, 0, :], r0[:, 0, :])
                    nc.gpsimd.partition_broadcast(bc[:, 1, :], r0[:, 1, :])
                    bcA = bc[:, 0, :]
                    bcB = bc[:, 1, :]
                    oT = pbo.tile([128, SC], mmdt, tag=f"oT{hp}{tag}", name=f"oT{hp}{tag}")
                    tmpB = pbt.tile([64, SC], mmdt, tag=f"tmpB{tag}", name=f"tmpB{tag}")
                    nc.vector.tensor_mul(oT[0:64, :], pva[0:64, :], bcA)
                    nc.vector.tensor_mul(tmpB, pvb[0:64, :], bcB)
                    nc.sync.dma_start(out=oT[64:128, :], in_=tmpB)
                    oTs.append(oT)

                for mt in range(8):
                    yps = pbps2.tile([128, SC], F32, tag=f"yps{tag}", name=f"yps{tag}", bufs=YBUFS)
                    ml = slice(mt * 128, (mt + 1) * 128)
                    for hp in range(4):
                        nc.tensor.matmul(yps, wo_sb[:, hp, ml], oTs[hp],
                                         start=(hp == 0), stop=(hp == 3))
                    ys = pbt.tile([128, SC], F32, tag=f"ys{tag}", name=f"ys{tag}")
                    nc.vector.tensor_copy(ys, yps)
                    nc.sync.dma_start(out=yT[ml, qsl], in_=ys)


_BUILT = {}


def build_nc(mmdt_name=MM_DTYPE, repeat=1):
    key = (mmdt_name, repeat)
    if key in _BUILT:
        return _BUILT[key]
    mmdt = {"f32": F32, "f32r": F32R}[mmdt_name]
    nc = bacc.Bacc("TRN2", target_bir_lowering=False, debug=False,
                   num_devices=N_CORES)
    dram = {
        "xT": nc.dram_tensor("xT", [D, S], mmdt, kind="ExternalInput").ap(),
        "wq": nc.dram_tensor("wq", [D, JC], mmdt, kind="ExternalInput").ap(),
        "wk": nc.dram_tensor("wk", [D, JC], mmdt, kind="ExternalInput").ap(),
        "wv": nc.dram_tensor("wv", [D, JC], mmdt, kind="ExternalInput").ap(),
        "wo": nc.dram_tensor("wo", [JC, D], mmdt, kind="ExternalInput").ap(),
        "cosE": nc.dram_tensor("cosE", [128, S], F32,
                               kind="ExternalInput").ap(),
        "sinE": nc.dram_tensor("sinE", [128, S], F32,
                               kind="ExternalInput").ap(),
        "maskneg": nc.dram_tensor("maskneg", [128, 128], F32,
                                  kind="ExternalInput").ap(),
        "yT": nc.dram_tensor("yT", [D, S], F32, kind="ExternalOutput").ap(),
    }
    with tile.TileContext(nc) as tc:
        for r in range(repeat):
            _emit(nc, tc, mmdt, dram, tag=f"r{r}" if repeat > 1 else "")
    nc.compile()
    _BUILT[key] = nc
    return nc


def _round_f32r(a):
    """Round-to-nearest onto the f32r grid (fp32 with low 12 mantissa bits 0)."""
    b = np.ascontiguousarray(a, np.float32).view(np.uint32).astype(np.uint64)
    b = (b + 0x800 + ((b >> 12) & 1)) & 0xFFFFF000
    return b.astype(np.uint32).view(np.float32)


def _host_prep(x, pos_ids, Wq, Wk, Wv, Wo, cos, sin, mmdt_name=None):
    """Build the 8 per-core input maps."""
    if mmdt_name is None:
        mmdt_name = MM_DTYPE
    rnd = _round_f32r if mmdt_name == "f32r" else (lambda a: a)
    x = np.asarray(x, dtype=np.float32)
    pos_ids = np.asarray(pos_ids)
    cos = np.asarray(cos, dtype=np.float32)
    sin = np.asarray(sin, dtype=np.float32)
    freq_idx = np.tile(np.repeat(np.arange(DK // 2), 2), 2)  # [128]
    sign = np.where((np.arange(128) % 2) == 0, -1.0, 1.0).astype(np.float32)

    # universal triangular boundary mask: 0 if q >= p else -1e30
    p = np.arange(128)[:, None]
    q = np.arange(128)[None, :]
    mask = np.where(q >= p, 0.0, -1e30).astype(np.float32)

    in_maps = []
    for c in range(N_CORES):
        b, g = c // 2, c % 2
        hs = slice(64 * HPC * g, 64 * HPC * g + JC)
        pos = pos_ids[b].astype(np.int64)
        cosT = cos[pos].T  # [32, S]
        sinT = sin[pos].T
        cosE = np.ascontiguousarray(cosT[freq_idx])           # [128, S]
        sinE = np.ascontiguousarray(sinT[freq_idx] * sign[:, None])
        in_maps.append({
            "xT": rnd(np.ascontiguousarray(x[b].T)),
            "wq": rnd(np.ascontiguousarray(Wq[hs, :].T)),
            "wk": rnd(np.ascontiguousarray(Wk[hs, :].T)),
            "wv": rnd(np.ascontiguousarray(Wv[hs, :].T)),
            "wo": rnd(np.ascontiguousarray(Wo[:, hs].T)),
            "cosE": cosE,
            "sinE": sinE,
            "maskneg": mask,
        })
    return in_maps


def kernel(x, pos_ids, Wq, Wk, Wv, Wo, cos, sin):
    nc = build_nc()
    in_maps = _host_prep(x, pos_ids, Wq, Wk, Wv, Wo, cos, sin)
    res = run_bass_kernel_spmd(nc, in_maps, list(range(N_CORES)))
    out = np.empty((B, S, D), dtype=np.float32)
    for b in range(B):
        yT = res.results[2 * b]["yT"] + res.results[2 * b + 1]["yT"]
        out[b] = yT.T
    return out


if __name__ == "__main__":
    t0 = time.time()
    nc = build_nc()
    print(f"build+compile: {time.time()-t0:.1f}s", flush=True)



# revision 6
# speedup vs baseline: 1.0740x; 1.0740x over previous
"""Causal multi-head self-attention with RoPE on 8 TRN2 NeuronCores.

Sharding: batch(4) x head-group(2) -> 8 cores. Core c handles batch c//2 and
heads [8*(c%2), 8*(c%2)+8). Each core computes its partial output projection
(sum over its 8 heads' contribution); the host adds the two head-group
partials per batch. No device collectives needed.

On-chip layout: sequence lives on the free dimension everywhere.
  - Q^T/K^T [j, s] come straight out of the projection matmuls
    (lhsT = W^T slices, rhs = x^T), RoPE applied with a partition pair-swap
    (stream_shuffle) + precomputed cos/sin tables. The RoPE elementwise work
    is split DVE (shuffle + sin-mul) / GpSimd (cos-mul + add) so phase A is
    PE-bound rather than DVE-bound.
  - scores are computed transposed S^T = K^T.T-contraction -> [k, q] tiles
    into a merged [128, 2, SC] PSUM tile (both heads of the pair), exp on
    ScalarE in one instruction per k-tile (no max subtraction needed:
    |scores| <= ~15), causal masking by adding -1e30 on diagonal tiles
    (GpSimd) before exp.
  - P@V contraction runs over k on partitions; a ones-row appended to V
    makes the softmax denominator fall out of the same matmul (M=65).
  - output projection contracts the 512 head-dims -> partial y^T [1024, s].

Engine budget per core (f32r, TimelineSim cost model): PE ~235us is the
floor; exp on ScalarE ~140us; DVE and GpSimd each well under; DMA ~90us.
Startup orders DMAs (wv, x-chunk first) so the first matmul starts ~9us in.
"""

import os
import sys
import time

for _p in ("/opt/trn_rl_repo", "/root/.axon_site/_ro/trn_rl_repo"):
    if _p not in sys.path and os.path.isdir(_p):
        sys.path.insert(0, _p)

import numpy as np
import concourse.bass as bass
import concourse.bacc as bacc
import concourse.mybir as mybir
import concourse.tile as tile
from concourse.bass_utils import run_bass_kernel_spmd

F32 = mybir.dt.float32
F32R = mybir.dt.float32r

B, S, D = 4, 2048, 1024
H, DK = 16, 64
HPC = 8            # heads per core
JC = HPC * DK      # 512 head-dims per core
N_CORES = 8
SC = 512           # q-chunk width (moving free dim)
NSC = S // SC      # 4
KT = 128           # k-tile (scores partition dim)
NKT = S // KT      # 16
DT = D // 128      # 8 contraction tiles for projections

# matmul operand dtype: "f32" (exact) or "f32r" (tf32-like, ~4x faster PE)
MM_DTYPE = os.environ.get("KERNEL_MM_DTYPE", "f32r")
EBUFS = int(os.environ.get("KV_EBUFS", "5"))
SCBUFS = int(os.environ.get("KV_SCBUFS", "2"))
YBUFS = int(os.environ.get("KV_YBUFS", "2"))
APBUFS = int(os.environ.get("KV_APBUFS", "3"))
XBUFS = int(os.environ.get("KV_XBUFS", "2"))
RBUFS = int(os.environ.get("KV_RBUFS", "2"))
OBUFS = int(os.environ.get("KV_OBUFS", "2"))
ROPE_SPLIT = os.environ.get("KV_ROPE_SPLIT", "1") == "1"
VCOPY_ACT = os.environ.get("KV_VCOPY_ACT", "1") == "1"
MASK_GPS = os.environ.get("KV_MASK_GPS", "1") == "1"
YS_GPS = os.environ.get("KV_YS_GPS", "1") == "1"


_PAIR_SWAP = []
for _i in range(16):
    _PAIR_SWAP += [2 * _i + 1, 2 * _i]


def _emit(nc, tc, mmdt, dram, tag=""):
    """Emit the whole per-core program. `dram` maps name -> DRAM AP."""
    xT = dram["xT"]
    wq, wk, wv, wo = dram["wq"], dram["wk"], dram["wv"], dram["wo"]
    cosE, sinE, maskneg = dram["cosE"], dram["sinE"], dram["maskneg"]
    yT = dram["yT"]

    need_round = mmdt != F32
    EXP = mybir.ActivationFunctionType.Exp

    import contextlib
    with contextlib.ExitStack() as ctx:
        # ---- persistent tiles -------------------------------------------
        per = ctx.enter_context(tc.tile_pool(name=f"per{tag}", bufs=1))
        QT = [per.tile([128, S], mmdt, tag=f"QT{j}{tag}", name=f"QT{j}{tag}") for j in range(4)]
        KTt = [per.tile([128, S], mmdt, tag=f"KT{j}{tag}", name=f"KT{j}{tag}") for j in range(4)]
        vo = [per.tile([128, HPC, 65], mmdt, tag=f"vo{i}{tag}", name=f"vo{i}{tag}") for i in range(NKT)]
        ones_sb = per.tile([128, HPC], F32, tag=f"ones{tag}", name=f"ones{tag}")
        cos_sb = per.tile([128, S], F32, tag=f"cos{tag}", name=f"cos{tag}")
        sin_sb = per.tile([128, S], F32, tag=f"sin{tag}", name=f"sin{tag}")
        mask_sb = per.tile([128, 2, 128], F32, tag=f"mask{tag}", name=f"mask{tag}")

        # ---- phase A: projections ---------------------------------------
        with tc.tile_pool(name=f"pA{tag}", bufs=1) as pa, \
             tc.tile_pool(name=f"pAx{tag}", bufs=XBUFS) as pax, \
             tc.tile_pool(name=f"pAt{tag}", bufs=RBUFS) as pat, \
             tc.tile_pool(name=f"pAps{tag}", bufs=APBUFS, space="PSUM") as paps:
            # DMA order matters: everything shares the DMA engine pool, so
            # issue what the first matmul needs (wv + x chunk 0) first.
            w_sb = {}
            for name, w_ap in (("wv", wv), ("wq", wq), ("wk", wk)):
                wt = pa.tile([128, DT, JC], mmdt, tag=f"{name}{tag}", name=f"{name}{tag}")
                w_sb[name] = wt

            xT_r = xT.rearrange("(dt p) s -> p dt s", p=128)

            def load_xc(sc):
                ssl = slice(sc * SC, (sc + 1) * SC)
                xc = pax.tile([128, DT, SC], mmdt, tag=f"xc{tag}", name=f"xc{tag}")
                for dt in range(DT):  # per-dt DMAs spread across queues
                    nc.sync.dma_start(out=xc[:, dt, :], in_=xT_r[:, dt, ssl])
                return xc

            def load_w(name, w_ap):
                wt = w_sb[name]
                w_r = w_ap.rearrange("(dt p) j -> p dt j", p=128)
                for dt in range(DT):
                    nc.sync.dma_start(out=wt[:, dt, :], in_=w_r[:, dt, :])

            load_w("wv", wv)
            xc_cur = load_xc(0)
            nc.sync.dma_start(out=cos_sb, in_=cosE)
            nc.sync.dma_start(out=sin_sb, in_=sinE)
            load_w("wq", wq)
            load_w("wk", wk)
            nc.sync.dma_start(out=mask_sb.rearrange("p a b -> p (a b)"),
                              in_=maskneg)
            nc.vector.memset(ones_sb, 1.0)
            # ones column of V is constant across the whole run: set it once.
            for i in range(NKT):
                o_dst = vo[i][:, :, 64:65]
                o_src = ones_sb.rearrange("p (h o) -> p h o", o=1)
                if need_round:
                    nc.scalar.copy(o_dst, o_src)
                else:
                    nc.vector.memset(o_dst, 1.0)

            for sc in range(NSC):
                ssl = slice(sc * SC, (sc + 1) * SC)
                xc = xc_cur

                # V: out[s, j] tiles, lhsT = x^T [d, s], rhs = Wv^T [d, j]
                for st in range(4):
                    pv = paps.tile([128, JC], F32, tag=f"vps{tag}", name=f"vps{tag}")
                    sl = slice(st * 128, (st + 1) * 128)
                    for dt in range(DT):
                        nc.tensor.matmul(
                            pv, xc[:, dt, sl], w_sb["wv"][:, dt, :],
                            start=(dt == 0), stop=(dt == DT - 1))
                    vt = vo[sc * 4 + st]
                    pv_r = pv.rearrange("p (h j) -> p h j", h=HPC)
                    if VCOPY_ACT:
                        nc.scalar.copy(vt[:, :, 0:64], pv_r)
                    else:
                        nc.vector.tensor_copy(vt[:, :, 0:64], pv_r)

                # prefetch next x chunk while computing on this one
                if sc + 1 < NSC:
                    xc_cur = load_xc(sc + 1)

                # Q/K: out = (W row-slice) @ x^T -> [j, s] + RoPE
                for wname, dst in (("wq", QT), ("wk", KTt)):
                    for jt in range(4):
                        ps = paps.tile([128, SC], F32, tag=f"qkps{tag}", name=f"qkps{tag}")
                        jl = slice(jt * 128, (jt + 1) * 128)
                        for dt in range(DT):
                            nc.tensor.matmul(
                                ps, w_sb[wname][:, dt, jl], xc[:, dt, :],
                                start=(dt == 0), stop=(dt == DT - 1))
                        # RoPE: dst = ps*cos + shuffle(ps)*sin.
                        # GpSimd cannot touch PSUM, so ScalarE stages ps into
                        # SBUF; DVE does shuffle + sin-mul, GpSimd cos-mul +
                        # the final add.
                        qs = pat.tile([128, SC], F32, tag=f"ropes{tag}", name=f"ropes{tag}")
                        qc_t = pat.tile([128, SC], F32, tag=f"ropec{tag}", name=f"ropec{tag}")
                        if ROPE_SPLIT:
                            pf = pat.tile([128, SC], F32, tag=f"ropef{tag}", name=f"ropef{tag}")
                            nc.scalar.copy(pf, ps)
                            nc.vector.stream_shuffle(qs, pf, _PAIR_SWAP)
                            nc.vector.tensor_mul(qs, qs, sin_sb[:, ssl])
                            nc.gpsimd.tensor_mul(qc_t, pf, cos_sb[:, ssl])
                            nc.gpsimd.tensor_add(dst[jt][:, ssl], qc_t, qs)
                        else:
                            nc.vector.stream_shuffle(qs, ps, _PAIR_SWAP)
                            nc.vector.tensor_mul(qs, qs, sin_sb[:, ssl])
                            nc.vector.tensor_mul(qc_t, ps, cos_sb[:, ssl])
                            nc.vector.tensor_add(dst[jt][:, ssl], qc_t, qs)

        # ---- phase B: attention + output projection ---------------------
        with tc.tile_pool(name=f"pB{tag}", bufs=1) as pb, \
             tc.tile_pool(name=f"pBe{tag}", bufs=EBUFS) as pbe, \
             tc.tile_pool(name=f"pBt{tag}", bufs=2) as pbt, \
             tc.tile_pool(name=f"pBo{tag}", bufs=OBUFS) as pbo, \
             tc.tile_pool(name=f"pBps{tag}", bufs=1, space="PSUM") as pbps, \
             tc.tile_pool(name=f"pBps2{tag}", bufs=2, space="PSUM") as pbps2:
            wo_sb = pb.tile([128, 4, D], mmdt, tag=f"wo{tag}", name=f"wo{tag}")
            wo_r = wo.rearrange("(hp p) m -> p hp m", p=128)
            for hp in range(4):
                nc.sync.dma_start(out=wo_sb[:, hp, :], in_=wo_r[:, hp, :])

            for qc in range(NSC):
                qsl = slice(qc * SC, (qc + 1) * SC)
                oTs = []
                for hp in range(4):
                    pva = pbps.tile([65, SC], F32, tag=f"pva{tag}", name=f"pva{tag}")
                    pvb = pbps.tile([65, SC], F32, tag=f"pvb{tag}", name=f"pvb{tag}")
                    nkt = 4 * qc + 4
                    h0, h1 = 2 * hp, 2 * hp + 1
                    pending = None  # software pipeline: PV lags scores by 1
                    for kt in range(nkt):
                        ksl = slice(kt * KT, (kt + 1) * KT)
                        d = kt - 4 * qc
                        # diagonal tiles: only columns q >= 128*d are causally
                        # valid -- shrink scores/exp/PV to that range; the
                        # boundary 128-wide strip still needs the triangular
                        # mask-add.
                        cs = 128 * d if d > 0 else 0
                        vq = slice(cs, SC)
                        qv = slice(qc * SC + cs, (qc + 1) * SC)
                        sc2 = pbps2.tile([128, 2, SC], F32, tag=f"sc2{tag}",
                                         name=f"sc2{tag}", bufs=SCBUFS)
                        sca, scb = sc2[:, 0, :], sc2[:, 1, :]
                        nc.tensor.matmul(sca[:, vq], KTt[hp][0:64, ksl],
                                         QT[hp][0:64, qv],
                                         start=True, stop=True,
                                         tile_position=(0, 0))
                        nc.tensor.matmul(scb[:, vq], KTt[hp][64:128, ksl],
                                         QT[hp][64:128, qv],
                                         start=True, stop=True,
                                         tile_position=(64, 0))
                        e2 = pbe.tile([128, 2, SC], mmdt, tag=f"e2{tag}",
                                      name=f"e2{tag}")
                        nc.scalar.activation(e2[:, :, vq], sc2[:, :, vq],
                                             EXP, scale=0.125)
                        if d >= 0:
                            # causal boundary strip: zero the exp of the
                            # invalid (q < k) positions with a 0/1 mask.
                            # GpSimd can do this since e2 lives in SBUF.
                            bs = slice(cs, cs + 128)
                            if MASK_GPS:
                                nc.gpsimd.tensor_mul(e2[:, :, bs],
                                                     e2[:, :, bs], mask_sb)
                            else:
                                nc.vector.tensor_mul(e2[:, :, bs],
                                                     e2[:, :, bs], mask_sb)
                        ea, eb = e2[:, 0, :], e2[:, 1, :]
                        if pending is not None:
                            pkt, pea, peb, pvq = pending
                            nc.tensor.matmul(pva[:, pvq], vo[pkt][:, h0, :],
                                             pea[:, pvq],
                                             start=(pkt == 0), stop=False)
                            nc.tensor.matmul(pvb[:, pvq], vo[pkt][:, h1, :],
                                             peb[:, pvq],
                                             start=(pkt == 0), stop=False)
                        pending = (kt, ea, eb, vq)
                    pkt, pea, peb, pvq = pending
                    nc.tensor.matmul(pva[:, pvq], vo[pkt][:, h0, :],
                                     pea[:, pvq],
                                     start=(pkt == 0), stop=True)
                    nc.tensor.matmul(pvb[:, pvq], vo[pkt][:, h1, :],
                                     peb[:, pvq],
                                     start=(pkt == 0), stop=True)
                    # normalize: oT[j, q] = pv[j, q] / denom[q].
                    # All DVE ops must be partition-aligned; the denom row
                    # lives at partition 64, so recip in place (64->64), DMA
                    # the row to partition 0, broadcast to 0:64, multiply at
                    # base 0, and DMA-relocate head B's rows to 64:128.
                    rc = pbt.tile([65, 2, SC], F32, tag=f"rc{tag}", name=f"rc{tag}", bufs=1)
                    nc.vector.reciprocal(rc[64:65, 0, :], pva[64:65, :])
                    nc.vector.reciprocal(rc[64:65, 1, :], pvb[64:65, :])
                    r0 = pbt.tile([1, 2, SC], F32, tag=f"r0{tag}", name=f"r0{tag}", bufs=1)
                    nc.sync.dma_start(out=r0, in_=rc[64:65, :, :])
                    bc = pbt.tile([64, 2, SC], F32, tag=f"bc{tag}", name=f"bc{tag}")
                    nc.gpsimd.partition_broadcast(bc[:, 0, :], r0[:, 0, :])
                    nc.gpsimd.partition_broadcast(bc[:, 1, :], r0[:, 1, :])
                    bcA = bc[:, 0, :]
                    bcB = bc[:, 1, :]
                    oT = pbo.tile([128, SC], mmdt, tag=f"oT{hp}{tag}", name=f"oT{hp}{tag}")
                    tmpB = pbt.tile([64, SC], mmdt, tag=f"tmpB{tag}", name=f"tmpB{tag}")
                    nc.vector.tensor_mul(oT[0:64, :], pva[0:64, :], bcA)
                    nc.vector.tensor_mul(tmpB, pvb[0:64, :], bcB)
                    nc.sync.dma_start(out=oT[64:128, :], in_=tmpB)
                    oTs.append(oT)

                for mt in range(8):
                    yps = pbps2.tile([128, SC], F32, tag=f"yps{tag}", name=f"yps{tag}", bufs=YBUFS)
                    ml = slice(mt * 128, (mt + 1) * 128)
                    for hp in range(4):
                        nc.tensor.matmul(yps, wo_sb[:, hp, ml], oTs[hp],
                                         start=(hp == 0), stop=(hp == 3))
                    ys = pbt.tile([128, SC], F32, tag=f"ys{tag}", name=f"ys{tag}")
                    nc.vector.tensor_copy(ys, yps)
                    nc.sync.dma_start(out=yT[ml, qsl], in_=ys)


_BUILT = {}


def build_nc(mmdt_name=MM_DTYPE, repeat=1):
    key = (mmdt_name, repeat)
    if key in _BUILT:
        return _BUILT[key]
    mmdt = {"f32": F32, "f32r": F32R}[mmdt_name]
    nc = bacc.Bacc("TRN2", target_bir_lowering=False, debug=False,
                   num_devices=N_CORES)
    dram = {
        "xT": nc.dram_tensor("xT", [D, S], mmdt, kind="ExternalInput").ap(),
        "wq": nc.dram_tensor("wq", [D, JC], mmdt, kind="ExternalInput").ap(),
        "wk": nc.dram_tensor("wk", [D, JC], mmdt, kind="ExternalInput").ap(),
        "wv": nc.dram_tensor("wv", [D, JC], mmdt, kind="ExternalInput").ap(),
        "wo": nc.dram_tensor("wo", [JC, D], mmdt, kind="ExternalInput").ap(),
        "cosE": nc.dram_tensor("cosE", [128, S], F32,
                               kind="ExternalInput").ap(),
        "sinE": nc.dram_tensor("sinE", [128, S], F32,
                               kind="ExternalInput").ap(),
        "maskneg": nc.dram_tensor("maskneg", [128, 256], F32,
                                  kind="ExternalInput").ap(),
        "yT": nc.dram_tensor("yT", [D, S], F32, kind="ExternalOutput").ap(),
    }
    with tile.TileContext(nc) as tc:
        for r in range(repeat):
            _emit(nc, tc, mmdt, dram, tag=f"r{r}" if repeat > 1 else "")
    nc.compile()
    _BUILT[key] = nc
    return nc


def _round_f32r(a):
    """Round-to-nearest onto the f32r grid (fp32 with low 12 mantissa bits 0)."""
    b = np.ascontiguousarray(a, np.float32).view(np.uint32).astype(np.uint64)
    b = (b + 0x800 + ((b >> 12) & 1)) & 0xFFFFF000
    return b.astype(np.uint32).view(np.float32)


def _host_prep(x, pos_ids, Wq, Wk, Wv, Wo, cos, sin, mmdt_name=None):
    """Build the 8 per-core input maps."""
    if mmdt_name is None:
        mmdt_name = MM_DTYPE
    rnd = _round_f32r if mmdt_name == "f32r" else (lambda a: a)
    x = np.asarray(x, dtype=np.float32)
    pos_ids = np.asarray(pos_ids)
    cos = np.asarray(cos, dtype=np.float32)
    sin = np.asarray(sin, dtype=np.float32)
    freq_idx = np.tile(np.repeat(np.arange(DK // 2), 2), 2)  # [128]
    sign = np.where((np.arange(128) % 2) == 0, -1.0, 1.0).astype(np.float32)

    # universal triangular boundary mask: 1 if q >= p else 0 (multiplied
    # into exp(scores) post-activation); two side-by-side copies (one per
    # head in the merged [128, 2, 128] tile)
    p = np.arange(128)[:, None]
    q = np.arange(128)[None, :]
    mask1 = np.where(q >= p, 1.0, 0.0).astype(np.float32)
    mask = np.concatenate([mask1, mask1], axis=1)  # [128, 256]

    in_maps = []
    for c in range(N_CORES):
        b, g = c // 2, c % 2
        hs = slice(64 * HPC * g, 64 * HPC * g + JC)
        pos = pos_ids[b].astype(np.int64)
        cosT = cos[pos].T  # [32, S]
        sinT = sin[pos].T
        cosE = np.ascontiguousarray(cosT[freq_idx])           # [128, S]
        sinE = np.ascontiguousarray(sinT[freq_idx] * sign[:, None])
        in_maps.append({
            "xT": rnd(np.ascontiguousarray(x[b].T)),
            "wq": rnd(np.ascontiguousarray(Wq[hs, :].T)),
            "wk": rnd(np.ascontiguousarray(Wk[hs, :].T)),
            "wv": rnd(np.ascontiguousarray(Wv[hs, :].T)),
            "wo": rnd(np.ascontiguousarray(Wo[:, hs].T)),
            "cosE": cosE,
            "sinE": sinE,
            "maskneg": mask,
        })
    return in_maps


def kernel(x, pos_ids, Wq, Wk, Wv, Wo, cos, sin):
    nc = build_nc()
    in_maps = _host_prep(x, pos_ids, Wq, Wk, Wv, Wo, cos, sin)
    res = run_bass_kernel_spmd(nc, in_maps, list(range(N_CORES)))
    out = np.empty((B, S, D), dtype=np.float32)
    for b in range(B):
        yT = res.results[2 * b]["yT"] + res.results[2 * b + 1]["yT"]
        out[b] = yT.T
    return out


if __name__ == "__main__":
    t0 = time.time()
    nc = build_nc()
    print(f"build+compile: {time.time()-t0:.1f}s", flush=True)


# revision 10
# speedup vs baseline: 1.2057x; 1.1226x over previous
"""Causal multi-head self-attention with RoPE on 8 TRN2 NeuronCores.

Sharding: batch(4) x head-group(2) -> 8 cores. Core c handles batch c//2 and
heads [8*(c%2), 8*(c%2)+8). Each core computes its partial output projection
(sum over its 8 heads' contribution); the host adds the two head-group
partials per batch. No device collectives needed.

On-chip layout: sequence lives on the free dimension everywhere.
  - Q^T/K^T [j, s] come straight out of the projection matmuls
    (lhsT = W^T slices, rhs = x^T), RoPE applied with a partition pair-swap
    (stream_shuffle) + precomputed cos/sin tables. The RoPE elementwise work
    is split DVE (shuffle + sin-mul) / GpSimd (cos-mul + add) so phase A is
    PE-bound rather than DVE-bound.
  - scores are computed transposed S^T = K^T.T-contraction -> [k, q] tiles
    into a merged [128, 2, SC] PSUM tile (both heads of the pair), exp on
    ScalarE in one instruction per k-tile (no max subtraction needed:
    |scores| <= ~15), causal masking by adding -1e30 on diagonal tiles
    (GpSimd) before exp.
  - P@V contraction runs over k on partitions; a ones-row appended to V
    makes the softmax denominator fall out of the same matmul (M=65).
  - output projection contracts the 512 head-dims -> partial y^T [1024, s].

Engine budget per core (f32r, TimelineSim cost model): PE ~235us is the
floor; exp on ScalarE ~140us; DVE and GpSimd each well under; DMA ~90us.
Startup orders DMAs (wv, x-chunk first) so the first matmul starts ~9us in.
"""

import os
import sys
import time

for _p in ("/opt/trn_rl_repo", "/root/.axon_site/_ro/trn_rl_repo"):
    if _p not in sys.path and os.path.isdir(_p):
        sys.path.insert(0, _p)

import numpy as np
import concourse.bass as bass
import concourse.bacc as bacc
import concourse.mybir as mybir
import concourse.tile as tile
from concourse.bass_utils import run_bass_kernel_spmd

F32 = mybir.dt.float32
F32R = mybir.dt.float32r

B, S, D = 4, 2048, 1024
H, DK = 16, 64
HPC = 8            # heads per core
JC = HPC * DK      # 512 head-dims per core
N_CORES = 8
SC = 512           # q-chunk width (moving free dim)
NSC = S // SC      # 4
KT = 128           # k-tile (scores partition dim)
NKT = S // KT      # 16
DT = D // 128      # 8 contraction tiles for projections

# matmul operand dtype: "f32" (exact) or "f32r" (tf32-like, ~4x faster PE)
MM_DTYPE = os.environ.get("KERNEL_MM_DTYPE", "f32r")
EBUFS = int(os.environ.get("KV_EBUFS", "5"))
SCBUFS = int(os.environ.get("KV_SCBUFS", "2"))
YBUFS = int(os.environ.get("KV_YBUFS", "2"))
APBUFS = int(os.environ.get("KV_APBUFS", "3"))
XBUFS = int(os.environ.get("KV_XBUFS", "2"))
RBUFS = int(os.environ.get("KV_RBUFS", "2"))
OBUFS = int(os.environ.get("KV_OBUFS", "2"))
ROPE_SPLIT = os.environ.get("KV_ROPE_SPLIT", "1") == "1"
VCOPY_ACT = os.environ.get("KV_VCOPY_ACT", "1") == "1"
MASK_GPS = os.environ.get("KV_MASK_GPS", "1") == "1"
YS_GPS = os.environ.get("KV_YS_GPS", "1") == "1"


_PAIR_SWAP = []
for _i in range(16):
    _PAIR_SWAP += [2 * _i + 1, 2 * _i]


def _emit(nc, tc, mmdt, dram, tag=""):
    """Emit the whole per-core program. `dram` maps name -> DRAM AP."""
    xT = dram["xT"]
    wq, wk, wv, wo = dram["wq"], dram["wk"], dram["wv"], dram["wo"]
    cosE, sinE, maskneg = dram["cosE"], dram["sinE"], dram["maskneg"]
    yT = dram["yT"]

    need_round = mmdt != F32
    EXP = mybir.ActivationFunctionType.Exp

    import contextlib
    with contextlib.ExitStack() as ctx:
        # ---- persistent tiles -------------------------------------------
        per = ctx.enter_context(tc.tile_pool(name=f"per{tag}", bufs=1))
        QT = [per.tile([128, S], mmdt, tag=f"QT{j}{tag}", name=f"QT{j}{tag}") for j in range(4)]
        KTt = [per.tile([128, S], mmdt, tag=f"KT{j}{tag}", name=f"KT{j}{tag}") for j in range(4)]
        vo = [per.tile([128, HPC, 65], mmdt, tag=f"vo{i}{tag}", name=f"vo{i}{tag}") for i in range(NKT)]
        ones_sb = per.tile([128, HPC], F32, tag=f"ones{tag}", name=f"ones{tag}")
        cos_sb = per.tile([128, S], F32, tag=f"cos{tag}", name=f"cos{tag}")
        sin_sb = per.tile([128, S], F32, tag=f"sin{tag}", name=f"sin{tag}")
        mask_sb = per.tile([128, 2, 128], F32, tag=f"mask{tag}", name=f"mask{tag}")

        # ---- phase A: projections ---------------------------------------
        with tc.tile_pool(name=f"pA{tag}", bufs=1) as pa, \
             tc.tile_pool(name=f"pAx{tag}", bufs=XBUFS) as pax, \
             tc.tile_pool(name=f"pAt{tag}", bufs=RBUFS) as pat, \
             tc.tile_pool(name=f"pAps{tag}", bufs=APBUFS, space="PSUM") as paps:
            # DMA order matters: everything shares the DMA engine pool, so
            # issue what the first matmul needs (wv + x chunk 0) first.
            w_sb = {}
            for name, w_ap in (("wv", wv), ("wq", wq), ("wk", wk)):
                wt = pa.tile([128, DT, JC], mmdt, tag=f"{name}{tag}", name=f"{name}{tag}")
                w_sb[name] = wt

            xT_r = xT.rearrange("(dt p) s -> p dt s", p=128)

            def load_xc(sc):
                ssl = slice(sc * SC, (sc + 1) * SC)
                xc = pax.tile([128, DT, SC], mmdt, tag=f"xc{tag}", name=f"xc{tag}")
                for dt in range(DT):  # per-dt DMAs spread across queues
                    nc.sync.dma_start(out=xc[:, dt, :], in_=xT_r[:, dt, ssl])
                return xc

            def load_w(name, w_ap):
                wt = w_sb[name]
                w_r = w_ap.rearrange("(dt p) j -> p dt j", p=128)
                for dt in range(DT):
                    nc.sync.dma_start(out=wt[:, dt, :], in_=w_r[:, dt, :])

            load_w("wv", wv)
            xc_cur = load_xc(0)
            nc.sync.dma_start(out=cos_sb, in_=cosE)
            nc.sync.dma_start(out=sin_sb, in_=sinE)
            load_w("wq", wq)
            load_w("wk", wk)
            nc.sync.dma_start(out=mask_sb.rearrange("p a b -> p (a b)"),
                              in_=maskneg)
            nc.vector.memset(ones_sb, 1.0)
            # ones column of V is constant across the whole run: set it once.
            for i in range(NKT):
                o_dst = vo[i][:, :, 64:65]
                o_src = ones_sb.rearrange("p (h o) -> p h o", o=1)
                if need_round:
                    nc.scalar.copy(o_dst, o_src)
                else:
                    nc.vector.memset(o_dst, 1.0)

            for sc in range(NSC):
                ssl = slice(sc * SC, (sc + 1) * SC)
                xc = xc_cur

                # V: out[s, j] tiles, lhsT = x^T [d, s], rhs = Wv^T [d, j]
                for st in range(4):
                    pv = paps.tile([128, JC], F32, tag=f"vps{tag}", name=f"vps{tag}")
                    sl = slice(st * 128, (st + 1) * 128)
                    for dt in range(DT):
                        nc.tensor.matmul(
                            pv, xc[:, dt, sl], w_sb["wv"][:, dt, :],
                            start=(dt == 0), stop=(dt == DT - 1))
                    vt = vo[sc * 4 + st]
                    pv_r = pv.rearrange("p (h j) -> p h j", h=HPC)
                    if VCOPY_ACT:
                        nc.scalar.copy(vt[:, :, 0:64], pv_r)
                    else:
                        nc.vector.tensor_copy(vt[:, :, 0:64], pv_r)

                # prefetch next x chunk while computing on this one
                if sc + 1 < NSC:
                    xc_cur = load_xc(sc + 1)

                # Q/K: out = (W row-slice) @ x^T -> [j, s] + RoPE
                for wname, dst in (("wq", QT), ("wk", KTt)):
                    for jt in range(4):
                        ps = paps.tile([128, SC], F32, tag=f"qkps{tag}", name=f"qkps{tag}")
                        jl = slice(jt * 128, (jt + 1) * 128)
                        for dt in range(DT):
                            nc.tensor.matmul(
                                ps, w_sb[wname][:, dt, jl], xc[:, dt, :],
                                start=(dt == 0), stop=(dt == DT - 1))
                        # RoPE: dst = ps*cos + shuffle(ps)*sin.
                        # GpSimd cannot touch PSUM, so ScalarE stages ps into
                        # SBUF; DVE does shuffle + sin-mul, GpSimd cos-mul +
                        # the final add.
                        qs = pat.tile([128, SC], F32, tag=f"ropes{tag}", name=f"ropes{tag}")
                        qc_t = pat.tile([128, SC], F32, tag=f"ropec{tag}", name=f"ropec{tag}")
                        if ROPE_SPLIT:
                            pf = pat.tile([128, SC], F32, tag=f"ropef{tag}", name=f"ropef{tag}")
                            nc.vector.tensor_copy(pf, ps)
                            nc.vector.stream_shuffle(qs, pf, _PAIR_SWAP)
                            nc.vector.tensor_mul(qs, qs, sin_sb[:, ssl])
                            nc.gpsimd.tensor_mul(qc_t, pf, cos_sb[:, ssl])
                            nc.gpsimd.tensor_add(dst[jt][:, ssl], qc_t, qs)
                        else:
                            nc.vector.stream_shuffle(qs, ps, _PAIR_SWAP)
                            nc.vector.tensor_mul(qs, qs, sin_sb[:, ssl])
                            nc.vector.tensor_mul(qc_t, ps, cos_sb[:, ssl])
                            nc.vector.tensor_add(dst[jt][:, ssl], qc_t, qs)

        # ---- phase B: attention + output projection ---------------------
        with tc.tile_pool(name=f"pB{tag}", bufs=1) as pb, \
             tc.tile_pool(name=f"pBe{tag}", bufs=EBUFS) as pbe, \
             tc.tile_pool(name=f"pBt{tag}", bufs=2) as pbt, \
             tc.tile_pool(name=f"pBo{tag}", bufs=OBUFS) as pbo, \
             tc.tile_pool(name=f"pBps{tag}", bufs=1, space="PSUM") as pbps, \
             tc.tile_pool(name=f"pBps2{tag}", bufs=2, space="PSUM") as pbps2:
            wo_sb = pb.tile([128, 4, D], mmdt, tag=f"wo{tag}", name=f"wo{tag}")
            wo_r = wo.rearrange("(hp p) m -> p hp m", p=128)
            for hp in range(4):
                nc.sync.dma_start(out=wo_sb[:, hp, :], in_=wo_r[:, hp, :])

            def emit_outproj(qsl_, oTs_, mts):
                for mt in mts:
                    yps = pbps2.tile([128, SC], F32, tag=f"yps{tag}",
                                     name=f"yps{tag}", bufs=YBUFS)
                    ml = slice(mt * 128, (mt + 1) * 128)
                    for hp_ in range(4):
                        nc.tensor.matmul(yps, wo_sb[:, hp_, ml], oTs_[hp_],
                                         start=(hp_ == 0), stop=(hp_ == 3))
                    ys = pbt.tile([128, SC], F32, tag=f"ys{tag}", name=f"ys{tag}")
                    nc.vector.tensor_copy(ys, yps)
                    nc.sync.dma_start(out=yT[ml, qsl_], in_=ys)

            prev = None  # (qsl, oTs) of the previous q-chunk, out-proj
            # deferred into the next chunk's hp loop so ready out-proj
            # matmuls fill PE while attention waits on exp/normalize.
            for qc in range(NSC):
                qsl = slice(qc * SC, (qc + 1) * SC)
                oTs = []
                for hp in range(4):
                    pva = pbps.tile([65, SC], F32, tag=f"pva{tag}", name=f"pva{tag}")
                    pvb = pbps.tile([65, SC], F32, tag=f"pvb{tag}", name=f"pvb{tag}")
                    nkt = 4 * qc + 4
                    h0, h1 = 2 * hp, 2 * hp + 1
                    pending = None  # software pipeline: PV lags scores by 1
                    for kt in range(nkt):
                        ksl = slice(kt * KT, (kt + 1) * KT)
                        d = kt - 4 * qc
                        # diagonal tiles: only columns q >= 128*d are causally
                        # valid -- shrink scores/exp/PV to that range; the
                        # boundary 128-wide strip still needs the triangular
                        # mask-add.
                        cs = 128 * d if d > 0 else 0
                        vq = slice(cs, SC)
                        qv = slice(qc * SC + cs, (qc + 1) * SC)
                        sc2 = pbps2.tile([128, 2, SC], F32, tag=f"sc2{tag}",
                                         name=f"sc2{tag}", bufs=SCBUFS)
                        sca, scb = sc2[:, 0, :], sc2[:, 1, :]
                        nc.tensor.matmul(sca[:, vq], KTt[hp][0:64, ksl],
                                         QT[hp][0:64, qv],
                                         start=True, stop=True,
                                         tile_position=(0, 0))
                        nc.tensor.matmul(scb[:, vq], KTt[hp][64:128, ksl],
                                         QT[hp][64:128, qv],
                                         start=True, stop=True,
                                         tile_position=(64, 0))
                        e2 = pbe.tile([128, 2, SC], mmdt, tag=f"e2{tag}",
                                      name=f"e2{tag}")
                        nc.scalar.activation(e2[:, :, vq], sc2[:, :, vq],
                                             EXP, scale=0.125)
                        if d >= 0:
                            # causal boundary strip: zero the exp of the
                            # invalid (q < k) positions with a 0/1 mask.
                            # GpSimd can do this since e2 lives in SBUF.
                            bs = slice(cs, cs + 128)
                            if MASK_GPS:
                                nc.gpsimd.tensor_mul(e2[:, :, bs],
                                                     e2[:, :, bs], mask_sb)
                            else:
                                nc.vector.tensor_mul(e2[:, :, bs],
                                                     e2[:, :, bs], mask_sb)
                        ea, eb = e2[:, 0, :], e2[:, 1, :]
                        if pending is not None:
                            pkt, pea, peb, pvq = pending
                            nc.tensor.matmul(pva[:, pvq], vo[pkt][:, h0, :],
                                             pea[:, pvq],
                                             start=(pkt == 0), stop=False)
                            nc.tensor.matmul(pvb[:, pvq], vo[pkt][:, h1, :],
                                             peb[:, pvq],
                                             start=(pkt == 0), stop=False)
                        pending = (kt, ea, eb, vq)
                    pkt, pea, peb, pvq = pending
                    nc.tensor.matmul(pva[:, pvq], vo[pkt][:, h0, :],
                                     pea[:, pvq],
                                     start=(pkt == 0), stop=True)
                    nc.tensor.matmul(pvb[:, pvq], vo[pkt][:, h1, :],
                                     peb[:, pvq],
                                     start=(pkt == 0), stop=True)
                    # normalize: oT[j, q] = pv[j, q] / denom[q].
                    # Stage PSUM -> SBUF first so pva/pvb free after one op
                    # each (the next hp's PV accumulation reuses the banks
                    # while the recip/broadcast/mul chain runs from SBUF).
                    o2 = pbt.tile([65, 2, SC], F32, tag=f"o2{tag}", name=f"o2{tag}")
                    nc.vector.tensor_copy(o2[:, 0, :], pva)
                    nc.vector.tensor_copy(o2[:, 1, :], pvb)
                    # All DVE ops must be partition-aligned; the denom row
                    # lives at partition 64, so recip in place (64->64), DMA
                    # the row to partition 0, broadcast to 0:64, multiply at
                    # base 0, and DMA-relocate head B's rows to 64:128.
                    nc.vector.reciprocal(o2[64:65, 0, :], o2[64:65, 0, :])
                    nc.vector.reciprocal(o2[64:65, 1, :], o2[64:65, 1, :])
                    r0 = pbt.tile([1, 2, SC], F32, tag=f"r0{tag}", name=f"r0{tag}", bufs=1)
                    nc.sync.dma_start(out=r0, in_=o2[64:65, :, :])
                    bc = pbt.tile([64, 2, SC], F32, tag=f"bc{tag}", name=f"bc{tag}")
                    nc.gpsimd.partition_broadcast(bc[:, 0, :], r0[:, 0, :])
                    nc.gpsimd.partition_broadcast(bc[:, 1, :], r0[:, 1, :])
                    bcA = bc[:, 0, :]
                    bcB = bc[:, 1, :]
                    oT = pbo.tile([128, SC], mmdt, tag=f"oT{hp}{tag}", name=f"oT{hp}{tag}")
                    tmpB = pbt.tile([64, SC], mmdt, tag=f"tmpB{tag}", name=f"tmpB{tag}")
                    nc.vector.tensor_mul(oT[0:64, :], o2[0:64, 0, :], bcA)
                    nc.vector.tensor_mul(tmpB, o2[0:64, 1, :], bcB)
                    nc.sync.dma_start(out=oT[64:128, :], in_=tmpB)
                    oTs.append(oT)
                    if prev is not None:
                        emit_outproj(prev[0], prev[1], [2 * hp, 2 * hp + 1])
                prev = (qsl, oTs)
            emit_outproj(prev[0], prev[1], range(8))


_BUILT = {}


def build_nc(mmdt_name=MM_DTYPE, repeat=1):
    key = (mmdt_name, repeat)
    if key in _BUILT:
        return _BUILT[key]
    mmdt = {"f32": F32, "f32r": F32R}[mmdt_name]
    nc = bacc.Bacc("TRN2", target_bir_lowering=False, debug=False,
                   num_devices=N_CORES)
    dram = {
        "xT": nc.dram_tensor("xT", [D, S], mmdt, kind="ExternalInput").ap(),
        "wq": nc.dram_tensor("wq", [D, JC], mmdt, kind="ExternalInput").ap(),
        "wk": nc.dram_tensor("wk", [D, JC], mmdt, kind="ExternalInput").ap(),
        "wv": nc.dram_tensor("wv", [D, JC], mmdt, kind="ExternalInput").ap(),
        "wo": nc.dram_tensor("wo", [JC, D], mmdt, kind="ExternalInput").ap(),
        "cosE": nc.dram_tensor("cosE", [128, S], F32,
                               kind="ExternalInput").ap(),
        "sinE": nc.dram_tensor("sinE", [128, S], F32,
                               kind="ExternalInput").ap(),
        "maskneg": nc.dram_tensor("maskneg", [128, 256], F32,
                                  kind="ExternalInput").ap(),
        "yT": nc.dram_tensor("yT", [D, S], F32, kind="ExternalOutput").ap(),
    }
    with tile.TileContext(nc) as tc:
        for r in range(repeat):
            _emit(nc, tc, mmdt, dram, tag=f"r{r}" if repeat > 1 else "")
    nc.compile()
    _BUILT[key] = nc
    return nc


def _round_f32r(a):
    """Round-to-nearest onto the f32r grid (fp32 with low 12 mantissa bits 0)."""
    b = np.ascontiguousarray(a, np.float32).view(np.uint32).astype(np.uint64)
    b = (b + 0x800 + ((b >> 12) & 1)) & 0xFFFFF000
    return b.astype(np.uint32).view(np.float32)


def _host_prep(x, pos_ids, Wq, Wk, Wv, Wo, cos, sin, mmdt_name=None):
    """Build the 8 per-core input maps."""
    if mmdt_name is None:
        mmdt_name = MM_DTYPE
    rnd = _round_f32r if mmdt_name == "f32r" else (lambda a: a)
    x = np.asarray(x, dtype=np.float32)
    pos_ids = np.asarray(pos_ids)
    cos = np.asarray(cos, dtype=np.float32)
    sin = np.asarray(sin, dtype=np.float32)
    freq_idx = np.tile(np.repeat(np.arange(DK // 2), 2), 2)  # [128]
    sign = np.where((np.arange(128) % 2) == 0, -1.0, 1.0).astype(np.float32)

    # universal triangular boundary mask: 1 if q >= p else 0 (multiplied
    # into exp(scores) post-activation); two side-by-side copies (one per
    # head in the merged [128, 2, 128] tile)
    p = np.arange(128)[:, None]
    q = np.arange(128)[None, :]
    mask1 = np.where(q >= p, 1.0, 0.0).astype(np.float32)
    mask = np.concatenate([mask1, mask1], axis=1)  # [128, 256]

    in_maps = []
    for c in range(N_CORES):
        b, g = c // 2, c % 2
        hs = slice(64 * HPC * g, 64 * HPC * g + JC)
        pos = pos_ids[b].astype(np.int64)
        cosT = cos[pos].T  # [32, S]
        sinT = sin[pos].T
        cosE = np.ascontiguousarray(cosT[freq_idx])           # [128, S]
        sinE = np.ascontiguousarray(sinT[freq_idx] * sign[:, None])
        in_maps.append({
            "xT": rnd(np.ascontiguousarray(x[b].T)),
            "wq": rnd(np.ascontiguousarray(Wq[hs, :].T)),
            "wk": rnd(np.ascontiguousarray(Wk[hs, :].T)),
            "wv": rnd(np.ascontiguousarray(Wv[hs, :].T)),
            "wo": rnd(np.ascontiguousarray(Wo[:, hs].T)),
            "cosE": cosE,
            "sinE": sinE,
            "maskneg": mask,
        })
    return in_maps


def kernel(x, pos_ids, Wq, Wk, Wv, Wo, cos, sin):
    nc = build_nc()
    in_maps = _host_prep(x, pos_ids, Wq, Wk, Wv, Wo, cos, sin)
    res = run_bass_kernel_spmd(nc, in_maps, list(range(N_CORES)))
    out = np.empty((B, S, D), dtype=np.float32)
    for b in range(B):
        yT = res.results[2 * b]["yT"] + res.results[2 * b + 1]["yT"]
        out[b] = yT.T
    return out


if __name__ == "__main__":
    t0 = time.time()
    nc = build_nc()
    print(f"build+compile: {time.time()-t0:.1f}s", flush=True)


# revision 12
# speedup vs baseline: 1.2122x; 1.0054x over previous
"""Causal multi-head self-attention with RoPE on 8 TRN2 NeuronCores.

Sharding: batch(4) x head-group(2) -> 8 cores. Core c handles batch c//2 and
heads [8*(c%2), 8*(c%2)+8). Each core computes its partial output projection
(sum over its 8 heads' contribution); the host adds the two head-group
partials per batch. No device collectives needed.

On-chip layout: sequence lives on the free dimension everywhere.
  - Q^T/K^T [j, s] come straight out of the projection matmuls
    (lhsT = W^T slices, rhs = x^T), RoPE applied with a partition pair-swap
    (stream_shuffle) + precomputed cos/sin tables. The RoPE elementwise work
    is split DVE (shuffle + sin-mul) / GpSimd (cos-mul + add) so phase A is
    PE-bound rather than DVE-bound.
  - scores are computed transposed S^T = K^T.T-contraction -> [k, q] tiles
    into a merged [128, 2, SC] PSUM tile (both heads of the pair), exp on
    ScalarE in one instruction per k-tile (no max subtraction needed:
    |scores| <= ~15), causal masking by adding -1e30 on diagonal tiles
    (GpSimd) before exp.
  - P@V contraction runs over k on partitions; a ones-row appended to V
    makes the softmax denominator fall out of the same matmul (M=65).
  - output projection contracts the 512 head-dims -> partial y^T [1024, s].

Engine budget per core (f32r, TimelineSim cost model): PE ~235us is the
floor; exp on ScalarE ~140us; DVE and GpSimd each well under; DMA ~90us.
Startup orders DMAs (wv, x-chunk first) so the first matmul starts ~9us in.
"""

import os
import sys
import time

for _p in ("/opt/trn_rl_repo", "/root/.axon_site/_ro/trn_rl_repo"):
    if _p not in sys.path and os.path.isdir(_p):
        sys.path.insert(0, _p)

import numpy as np
import concourse.bass as bass
import concourse.bacc as bacc
import concourse.mybir as mybir
import concourse.tile as tile
from concourse.bass_utils import run_bass_kernel_spmd

F32 = mybir.dt.float32
F32R = mybir.dt.float32r

B, S, D = 4, 2048, 1024
H, DK = 16, 64
HPC = 8            # heads per core
JC = HPC * DK      # 512 head-dims per core
N_CORES = 8
SC = 512           # q-chunk width (moving free dim)
NSC = S // SC      # 4
KT = 128           # k-tile (scores partition dim)
NKT = S // KT      # 16
DT = D // 128      # 8 contraction tiles for projections

# matmul operand dtype: "f32" (exact) or "f32r" (tf32-like, ~4x faster PE)
MM_DTYPE = os.environ.get("KERNEL_MM_DTYPE", "f32r")
EBUFS = int(os.environ.get("KV_EBUFS", "5"))
SCBUFS = int(os.environ.get("KV_SCBUFS", "2"))
YBUFS = int(os.environ.get("KV_YBUFS", "2"))
APBUFS = int(os.environ.get("KV_APBUFS", "3"))
XBUFS = int(os.environ.get("KV_XBUFS", "2"))
RBUFS = int(os.environ.get("KV_RBUFS", "2"))
OBUFS = int(os.environ.get("KV_OBUFS", "2"))
ROPE_SPLIT = os.environ.get("KV_ROPE_SPLIT", "1") == "1"
VCOPY_ACT = os.environ.get("KV_VCOPY_ACT", "1") == "1"
MASK_GPS = os.environ.get("KV_MASK_GPS", "1") == "1"
YS_GPS = os.environ.get("KV_YS_GPS", "1") == "1"


_PAIR_SWAP = []
for _i in range(16):
    _PAIR_SWAP += [2 * _i + 1, 2 * _i]


def _emit(nc, tc, mmdt, dram, tag=""):
    """Emit the whole per-core program. `dram` maps name -> DRAM AP."""
    xT = dram["xT"]
    wq, wk, wv, wo = dram["wq"], dram["wk"], dram["wv"], dram["wo"]
    cosE, sinE, maskneg = dram["cosE"], dram["sinE"], dram["maskneg"]
    yT = dram["yT"]

    need_round = mmdt != F32
    EXP = mybir.ActivationFunctionType.Exp

    import contextlib
    with contextlib.ExitStack() as ctx:
        # ---- persistent tiles -------------------------------------------
        per = ctx.enter_context(tc.tile_pool(name=f"per{tag}", bufs=1))
        QT = [per.tile([128, S], mmdt, tag=f"QT{j}{tag}", name=f"QT{j}{tag}") for j in range(4)]
        KTt = [per.tile([128, S], mmdt, tag=f"KT{j}{tag}", name=f"KT{j}{tag}") for j in range(4)]
        vo = [per.tile([128, HPC, 65], mmdt, tag=f"vo{i}{tag}", name=f"vo{i}{tag}") for i in range(NKT)]
        ones_sb = per.tile([128, HPC], F32, tag=f"ones{tag}", name=f"ones{tag}")
        cos_sb = per.tile([128, S], F32, tag=f"cos{tag}", name=f"cos{tag}")
        sin_sb = per.tile([128, S], F32, tag=f"sin{tag}", name=f"sin{tag}")
        mask_sb = per.tile([128, 2, 128], F32, tag=f"mask{tag}", name=f"mask{tag}")

        # ---- phase A: projections ---------------------------------------
        with tc.tile_pool(name=f"pA{tag}", bufs=1) as pa, \
             tc.tile_pool(name=f"pAx{tag}", bufs=XBUFS) as pax, \
             tc.tile_pool(name=f"pAt{tag}", bufs=RBUFS) as pat, \
             tc.tile_pool(name=f"pAps{tag}", bufs=APBUFS, space="PSUM") as paps:
            # DMA order matters: everything shares the DMA engine pool, so
            # issue what the first matmul needs (wv + x chunk 0) first.
            w_sb = {}
            for name, w_ap in (("wv", wv), ("wq", wq), ("wk", wk)):
                wt = pa.tile([128, DT, JC], mmdt, tag=f"{name}{tag}", name=f"{name}{tag}")
                w_sb[name] = wt

            xT_r = xT.rearrange("(dt p) s -> p dt s", p=128)

            def load_xc(sc):
                ssl = slice(sc * SC, (sc + 1) * SC)
                xc = pax.tile([128, DT, SC], mmdt, tag=f"xc{tag}", name=f"xc{tag}")
                for dt in range(DT):  # per-dt DMAs spread across queues
                    nc.sync.dma_start(out=xc[:, dt, :], in_=xT_r[:, dt, ssl])
                return xc

            def load_w(name, w_ap):
                wt = w_sb[name]
                w_r = w_ap.rearrange("(dt p) j -> p dt j", p=128)
                for dt in range(DT):
                    nc.sync.dma_start(out=wt[:, dt, :], in_=w_r[:, dt, :])

            load_w("wv", wv)
            xc_cur = load_xc(0)
            nc.sync.dma_start(out=cos_sb, in_=cosE)
            nc.sync.dma_start(out=sin_sb, in_=sinE)
            load_w("wq", wq)
            load_w("wk", wk)
            nc.sync.dma_start(out=mask_sb.rearrange("p a b -> p (a b)"),
                              in_=maskneg)
            nc.vector.memset(ones_sb, 1.0)
            # ones column of V is constant across the whole run: set it once.
            for i in range(NKT):
                o_dst = vo[i][:, :, 64:65]
                o_src = ones_sb.rearrange("p (h o) -> p h o", o=1)
                if need_round:
                    nc.scalar.copy(o_dst, o_src)
                else:
                    nc.vector.memset(o_dst, 1.0)

            for sc in range(NSC):
                ssl = slice(sc * SC, (sc + 1) * SC)
                xc = xc_cur

                # V: out[s, j] tiles, lhsT = x^T [d, s], rhs = Wv^T [d, j]
                for st in range(4):
                    pv = paps.tile([128, JC], F32, tag=f"vps{tag}", name=f"vps{tag}")
                    sl = slice(st * 128, (st + 1) * 128)
                    for dt in range(DT):
                        nc.tensor.matmul(
                            pv, xc[:, dt, sl], w_sb["wv"][:, dt, :],
                            start=(dt == 0), stop=(dt == DT - 1))
                    vt = vo[sc * 4 + st]
                    pv_r = pv.rearrange("p (h j) -> p h j", h=HPC)
                    if VCOPY_ACT:
                        nc.scalar.copy(vt[:, :, 0:64], pv_r)
                    else:
                        nc.vector.tensor_copy(vt[:, :, 0:64], pv_r)

                # prefetch next x chunk while computing on this one
                if sc + 1 < NSC:
                    xc_cur = load_xc(sc + 1)

                # Q/K: out = (W row-slice) @ x^T -> [j, s] + RoPE
                for wname, dst in (("wq", QT), ("wk", KTt)):
                    for jt in range(4):
                        ps = paps.tile([128, SC], F32, tag=f"qkps{tag}", name=f"qkps{tag}")
                        jl = slice(jt * 128, (jt + 1) * 128)
                        for dt in range(DT):
                            nc.tensor.matmul(
                                ps, w_sb[wname][:, dt, jl], xc[:, dt, :],
                                start=(dt == 0), stop=(dt == DT - 1))
                        # RoPE: dst = ps*cos + shuffle(ps)*sin.
                        # GpSimd cannot touch PSUM, so ScalarE stages ps into
                        # SBUF; DVE does shuffle + sin-mul, GpSimd cos-mul +
                        # the final add.
                        qs = pat.tile([128, SC], F32, tag=f"ropes{tag}", name=f"ropes{tag}")
                        qc_t = pat.tile([128, SC], F32, tag=f"ropec{tag}", name=f"ropec{tag}")
                        if ROPE_SPLIT:
                            pf = pat.tile([128, SC], F32, tag=f"ropef{tag}", name=f"ropef{tag}")
                            nc.vector.tensor_copy(pf, ps)
                            nc.vector.stream_shuffle(qs, pf, _PAIR_SWAP)
                            nc.vector.tensor_mul(qs, qs, sin_sb[:, ssl])
                            nc.gpsimd.tensor_mul(qc_t, pf, cos_sb[:, ssl])
                            nc.gpsimd.tensor_add(dst[jt][:, ssl], qc_t, qs)
                        else:
                            nc.vector.stream_shuffle(qs, ps, _PAIR_SWAP)
                            nc.vector.tensor_mul(qs, qs, sin_sb[:, ssl])
                            nc.vector.tensor_mul(qc_t, ps, cos_sb[:, ssl])
                            nc.vector.tensor_add(dst[jt][:, ssl], qc_t, qs)

        # ---- phase B: attention + output projection ---------------------
        with tc.tile_pool(name=f"pB{tag}", bufs=1) as pb, \
             tc.tile_pool(name=f"pBe{tag}", bufs=EBUFS) as pbe, \
             tc.tile_pool(name=f"pBt{tag}", bufs=2) as pbt, \
             tc.tile_pool(name=f"pBo{tag}", bufs=OBUFS) as pbo, \
             tc.tile_pool(name=f"pBps{tag}", bufs=1, space="PSUM") as pbps, \
             tc.tile_pool(name=f"pBps2{tag}", bufs=2, space="PSUM") as pbps2:
            wo_sb = pb.tile([128, 4, D], mmdt, tag=f"wo{tag}", name=f"wo{tag}")
            wo_r = wo.rearrange("(hp p) m -> p hp m", p=128)
            for hp in range(4):
                nc.sync.dma_start(out=wo_sb[:, hp, :], in_=wo_r[:, hp, :])

            def emit_outproj(qsl_, oTs_, mts):
                for mt in mts:
                    yps = pbps2.tile([128, SC], F32, tag=f"yps{tag}",
                                     name=f"yps{tag}", bufs=YBUFS)
                    ml = slice(mt * 128, (mt + 1) * 128)
                    for hp_ in range(4):
                        nc.tensor.matmul(yps, wo_sb[:, hp_, ml], oTs_[hp_],
                                         start=(hp_ == 0), stop=(hp_ == 3))
                    ys = pbt.tile([128, SC], F32, tag=f"ys{tag}", name=f"ys{tag}")
                    nc.vector.tensor_copy(ys, yps)
                    nc.sync.dma_start(out=yT[ml, qsl_], in_=ys)

            prev = None  # (qsl, oTs) of the previous q-chunk, out-proj
            # deferred into the next chunk's hp loop so ready out-proj
            # matmuls fill PE while attention waits on exp/normalize.
            for qc in range(NSC):
                qsl = slice(qc * SC, (qc + 1) * SC)
                oTs = []
                for hp in range(4):
                    pva = pbps.tile([65, SC], F32, tag=f"pva{tag}", name=f"pva{tag}")
                    pvb = pbps.tile([65, SC], F32, tag=f"pvb{tag}", name=f"pvb{tag}")
                    nkt = 4 * qc + 4
                    h0, h1 = 2 * hp, 2 * hp + 1
                    pending = None  # software pipeline: PV lags scores by 1
                    for kt in range(nkt):
                        ksl = slice(kt * KT, (kt + 1) * KT)
                        d = kt - 4 * qc
                        # diagonal tiles: only columns q >= 128*d are causally
                        # valid -- shrink scores/exp/PV to that range; the
                        # boundary 128-wide strip still needs the triangular
                        # mask-add.
                        cs = 128 * d if d > 0 else 0
                        vq = slice(cs, SC)
                        qv = slice(qc * SC + cs, (qc + 1) * SC)
                        sc2 = pbps2.tile([128, 2, SC], F32, tag=f"sc2{tag}",
                                         name=f"sc2{tag}", bufs=SCBUFS)
                        sca, scb = sc2[:, 0, :], sc2[:, 1, :]
                        nc.tensor.matmul(sca[:, vq], KTt[hp][0:64, ksl],
                                         QT[hp][0:64, qv],
                                         start=True, stop=True,
                                         tile_position=(0, 0))
                        nc.tensor.matmul(scb[:, vq], KTt[hp][64:128, ksl],
                                         QT[hp][64:128, qv],
                                         start=True, stop=True,
                                         tile_position=(64, 0))
                        e2 = pbe.tile([128, 2, SC], mmdt, tag=f"e2{tag}",
                                      name=f"e2{tag}")
                        nc.scalar.activation(e2[:, :, vq], sc2[:, :, vq],
                                             EXP, scale=0.125)
                        if d >= 0:
                            # causal boundary strip: zero the exp of the
                            # invalid (q < k) positions with a 0/1 mask.
                            # GpSimd can do this since e2 lives in SBUF.
                            bs = slice(cs, cs + 128)
                            if MASK_GPS:
                                nc.gpsimd.tensor_mul(e2[:, :, bs],
                                                     e2[:, :, bs], mask_sb)
                            else:
                                nc.vector.tensor_mul(e2[:, :, bs],
                                                     e2[:, :, bs], mask_sb)
                        ea, eb = e2[:, 0, :], e2[:, 1, :]
                        if pending is not None:
                            pkt, pea, peb, pvq = pending
                            nc.tensor.matmul(pva[:, pvq], vo[pkt][:, h0, :],
                                             pea[:, pvq],
                                             start=(pkt == 0), stop=False)
                            nc.tensor.matmul(pvb[:, pvq], vo[pkt][:, h1, :],
                                             peb[:, pvq],
                                             start=(pkt == 0), stop=False)
                        pending = (kt, ea, eb, vq)
                    pkt, pea, peb, pvq = pending
                    nc.tensor.matmul(pva[:, pvq], vo[pkt][:, h0, :],
                                     pea[:, pvq],
                                     start=(pkt == 0), stop=True)
                    nc.tensor.matmul(pvb[:, pvq], vo[pkt][:, h1, :],
                                     peb[:, pvq],
                                     start=(pkt == 0), stop=True)
                    # normalize: oT[j, q] = pv[j, q] / denom[q].
                    # Stage PSUM -> SBUF (partition-aligned) so pva/pvb free
                    # early; reciprocal reads the PSUM denom row directly
                    # (in@p64 -> out@p0 is valid for single-input DVE ops);
                    # broadcast to 64 partitions at base 0; head B's rows are
                    # DMA-relocated to 64:128 (engine ops cannot cross-base).
                    o2 = pbt.tile([65, 2, SC], F32, tag=f"o2{tag}", name=f"o2{tag}")
                    nc.vector.tensor_copy(o2[:, 0, :], pva)
                    nc.vector.tensor_copy(o2[:, 1, :], pvb)
                    d2a = pbt.tile([1, SC], F32, tag=f"d2a{tag}", name=f"d2a{tag}")
                    d2b = pbt.tile([1, SC], F32, tag=f"d2b{tag}", name=f"d2b{tag}")
                    nc.vector.reciprocal(d2a, pva[64:65, :])
                    nc.vector.reciprocal(d2b, pvb[64:65, :])
                    bc = pbt.tile([64, 2, SC], F32, tag=f"bc{tag}", name=f"bc{tag}")
                    nc.gpsimd.partition_broadcast(bc[:, 0, :], d2a)
                    nc.gpsimd.partition_broadcast(bc[:, 1, :], d2b)
                    oT = pbo.tile([128, SC], mmdt, tag=f"oT{hp}{tag}", name=f"oT{hp}{tag}")
                    tmpB = pbt.tile([64, SC], mmdt, tag=f"tmpB{tag}", name=f"tmpB{tag}")
                    nc.vector.tensor_mul(oT[0:64, :], o2[0:64, 0, :], bc[:, 0, :])
                    nc.vector.tensor_mul(tmpB, o2[0:64, 1, :], bc[:, 1, :])
                    nc.sync.dma_start(out=oT[64:128, :], in_=tmpB)
                    oTs.append(oT)
                    if prev is not None:
                        emit_outproj(prev[0], prev[1], [2 * hp, 2 * hp + 1])
                prev = (qsl, oTs)
            emit_outproj(prev[0], prev[1], range(8))


_BUILT = {}


def build_nc(mmdt_name=MM_DTYPE, repeat=1):
    key = (mmdt_name, repeat)
    if key in _BUILT:
        return _BUILT[key]
    mmdt = {"f32": F32, "f32r": F32R}[mmdt_name]
    nc = bacc.Bacc("TRN2", target_bir_lowering=False, debug=False,
                   num_devices=N_CORES)
    dram = {
        "xT": nc.dram_tensor("xT", [D, S], mmdt, kind="ExternalInput").ap(),
        "wq": nc.dram_tensor("wq", [D, JC], mmdt, kind="ExternalInput").ap(),
        "wk": nc.dram_tensor("wk", [D, JC], mmdt, kind="ExternalInput").ap(),
        "wv": nc.dram_tensor("wv", [D, JC], mmdt, kind="ExternalInput").ap(),
        "wo": nc.dram_tensor("wo", [JC, D], mmdt, kind="ExternalInput").ap(),
        "cosE": nc.dram_tensor("cosE", [128, S], F32,
                               kind="ExternalInput").ap(),
        "sinE": nc.dram_tensor("sinE", [128, S], F32,
                               kind="ExternalInput").ap(),
        "maskneg": nc.dram_tensor("maskneg", [128, 256], F32,
                                  kind="ExternalInput").ap(),
        "yT": nc.dram_tensor("yT", [D, S], F32, kind="ExternalOutput").ap(),
    }
    with tile.TileContext(nc) as tc:
        for r in range(repeat):
            _emit(nc, tc, mmdt, dram, tag=f"r{r}" if repeat > 1 else "")
    nc.compile()
    _BUILT[key] = nc
    return nc


def _round_f32r(a):
    """Round-to-nearest onto the f32r grid (fp32 with low 12 mantissa bits 0)."""
    b = np.ascontiguousarray(a, np.float32).view(np.uint32).astype(np.uint64)
    b = (b + 0x800 + ((b >> 12) & 1)) & 0xFFFFF000
    return b.astype(np.uint32).view(np.float32)


def _host_prep(x, pos_ids, Wq, Wk, Wv, Wo, cos, sin, mmdt_name=None):
    """Build the 8 per-core input maps."""
    if mmdt_name is None:
        mmdt_name = MM_DTYPE
    rnd = _round_f32r if mmdt_name == "f32r" else (lambda a: a)
    x = np.asarray(x, dtype=np.float32)
    pos_ids = np.asarray(pos_ids)
    cos = np.asarray(cos, dtype=np.float32)
    sin = np.asarray(sin, dtype=np.float32)
    freq_idx = np.tile(np.repeat(np.arange(DK // 2), 2), 2)  # [128]
    sign = np.where((np.arange(128) % 2) == 0, -1.0, 1.0).astype(np.float32)

    # universal triangular boundary mask: 1 if q >= p else 0 (multiplied
    # into exp(scores) post-activation); two side-by-side copies (one per
    # head in the merged [128, 2, 128] tile)
    p = np.arange(128)[:, None]
    q = np.arange(128)[None, :]
    mask1 = np.where(q >= p, 1.0, 0.0).astype(np.float32)
    mask = np.concatenate([mask1, mask1], axis=1)  # [128, 256]

    in_maps = []
    for c in range(N_CORES):
        b, g = c // 2, c % 2
        hs = slice(64 * HPC * g, 64 * HPC * g + JC)
        pos = pos_ids[b].astype(np.int64)
        cosT = cos[pos].T  # [32, S]
        sinT = sin[pos].T
        cosE = np.ascontiguousarray(cosT[freq_idx])           # [128, S]
        sinE = np.ascontiguousarray(sinT[freq_idx] * sign[:, None])
        in_maps.append({
            "xT": rnd(np.ascontiguousarray(x[b].T)),
            "wq": rnd(np.ascontiguousarray(Wq[hs, :].T)),
            "wk": rnd(np.ascontiguousarray(Wk[hs, :].T)),
            "wv": rnd(np.ascontiguousarray(Wv[hs, :].T)),
            "wo": rnd(np.ascontiguousarray(Wo[:, hs].T)),
            "cosE": cosE,
            "sinE": sinE,
            "maskneg": mask,
        })
    return in_maps


def kernel(x, pos_ids, Wq, Wk, Wv, Wo, cos, sin):
    nc = build_nc()
    in_maps = _host_prep(x, pos_ids, Wq, Wk, Wv, Wo, cos, sin)
    res = run_bass_kernel_spmd(nc, in_maps, list(range(N_CORES)))
    out = np.empty((B, S, D), dtype=np.float32)
    for b in range(B):
        yT = res.results[2 * b]["yT"] + res.results[2 * b + 1]["yT"]
        out[b] = yT.T
    return out


if __name__ == "__main__":
    t0 = time.time()
    nc = build_nc()
    print(f"build+compile: {time.time()-t0:.1f}s", flush=True)
